# revision 34
# baseline (speedup 1.0000x reference)
"""Causal self-attention (B=4, T=2048, D=1024, H=16) on 8 trn2 NeuronCores.

Sharding: Megatron-style tensor parallel over heads (TP=2) x data parallel
over batch (DP=4). Core c handles batch c//2 and head-group c%2 (8 heads).
Each core computes its QKV projection slice, causal attention for its 8
heads, and a partial output projection; the host sums the two TP partials
per batch and adds b_proj (plus the folded v-bias contribution).

Schedule: single software-pipelined stream.  Attention for query block q0
is exp-rate-limited on the scalar engine, so independent projection
matmuls (QKV of block q0+1, output projection of earlier blocks) are
interleaved into the attention instruction stream as PE filler.  The v
bias is folded into a host-side output correction (attn rows sum to 1),
and the output partials are written as fp16 to halve the drain DMA.
"""
import sys
from collections import deque

sys.path.insert(0, "/opt/trn_rl_repo")

import numpy as np
import ml_dtypes

import concourse.bass as bass
import concourse.tile as tile
from concourse import bacc, mybir
from concourse.bass_utils import run_bass_kernel_spmd

B, T, D, H = 4, 2048, 1024, 16
HD = 64            # head dim
HL = 8             # heads per core (TP=2)
DL = HL * HD       # 512 local qkv width
KCH = D // 128     # 8 contraction chunks
TCH = T // 128     # 16 token tiles of 128
NB = T // 512      # 4 query blocks of 512
F16 = mybir.dt.float16
F32 = mybir.dt.float32
NEG = -1.0e30

_cache = {}


def _build():
    nc = bacc.Bacc("TRN2", target_bir_lowering=False, num_devices=8)

    xT = nc.dram_tensor("xT", [D, T], F16, kind="ExternalInput")
    wqk = nc.dram_tensor("wqk", [D, 2 * DL], F16, kind="ExternalInput")
    bqk = nc.dram_tensor("bqk", [128, 2 * DL // 128], F32, kind="ExternalInput")
    wv = nc.dram_tensor("wv", [D, DL], F16, kind="ExternalInput")
    wp = nc.dram_tensor("wp", [DL, D], F16, kind="ExternalInput")
    tri = nc.dram_tensor("tri", [128, 128], F32, kind="ExternalInput")
    out = nc.dram_tensor("out", [T, D], F16, kind="ExternalOutput")

    with tile.TileContext(nc) as tc:
        with (
            tc.tile_pool(name="const", bufs=1) as const,
            tc.tile_pool(name="acts", bufs=1) as acts,
            tc.tile_pool(name="work", bufs=5) as work,
            tc.tile_pool(name="small", bufs=6) as small,
            tc.tile_pool(name="outp", bufs=4) as outp,
            tc.tile_pool(name="psS", bufs=2, space="PSUM") as psS,
            tc.tile_pool(name="psY", bufs=2, space="PSUM") as psY,
            tc.tile_pool(name="psF", bufs=2, space="PSUM") as psF,
        ):
            # ---- SBUF residents ----
            xT_sb = [const.tile([128, T], F16, name=f"xT{k}", tag=f"xT{k}")
                     for k in range(KCH)]
            wqk_sb = [const.tile([128, 2 * DL], F16, name=f"wqk{k}",
                                 tag=f"wqk{k}") for k in range(KCH)]
            wv_sb = [const.tile([128, DL], F16, name=f"wv{k}", tag=f"wv{k}")
                     for k in range(KCH)]
            wp_sb = [const.tile([128, D], F16, name=f"wp{c}", tag=f"wp{c}")
                     for c in range(DL // 128)]
            bqk_sb = const.tile([128, 2 * DL // 128], F32)
            tri_sb = const.tile([128, 128], F32)

            qT_sb = [acts.tile([128, T], F16, name=f"qT{c}", tag=f"qT{c}")
                     for c in range(4)]
            # kT per head, zero-padded to K=128: head 2c in partitions 0:64
            # of kT2[2c], head 2c+1 in partitions 64:128 of kT2[2c+1]; the
            # other half stays zero so S matmuls run full-K (no row-group
            # masking -> PE clock stays at full rate).
            kT2_sb = [acts.tile([128, T], F16, name=f"kT2h{h}", tag=f"kT2h{h}")
                      for h in range(HL)]
            vaug = [acts.tile([128, HL * (HD + 1)], F16, name=f"va{t}",
                              tag=f"va{t}") for t in range(TCH)]
            yT_sb = [acts.tile([128, T], F16, name=f"yT{c}", tag=f"yT{c}")
                     for c in range(4)]

            # ---- input DMAs on 3 queues, in consumption order ----
            # The v chains of block 0 are the first consumers: they need
            # xT t-tile 0 (all k chunks) + wv, so those go first at fine
            # granularity.  sync: xT; scalar: wqk; gpsimd: wv/bqk/tri/wp.
            nc.gpsimd.dma_start(out=bqk_sb, in_=bqk[:, :])
            nc.gpsimd.dma_start(out=tri_sb, in_=tri[:, :])
            for t in range(4):
                for k in range(KCH):
                    nc.sync.dma_start(
                        out=xT_sb[k][:, 128 * t:128 * (t + 1)],
                        in_=xT[128 * k:128 * (k + 1), 128 * t:128 * (t + 1)])
            for k in range(KCH):
                nc.scalar.dma_start(out=wv_sb[k],
                                    in_=wv[128 * k:128 * (k + 1), :])
            for k in range(KCH):
                nc.scalar.dma_start(out=wqk_sb[k][:, 0:128],
                                    in_=wqk[128 * k:128 * (k + 1), 0:128])
                nc.scalar.dma_start(out=wqk_sb[k][:, 512:640],
                                    in_=wqk[128 * k:128 * (k + 1), 512:640])
            for k in range(KCH):
                nc.sync.dma_start(out=xT_sb[k][:, 512:1024],
                                  in_=xT[128 * k:128 * (k + 1), 512:1024])
                nc.scalar.dma_start(out=wqk_sb[k][:, 128:512],
                                    in_=wqk[128 * k:128 * (k + 1), 128:512])
                nc.scalar.dma_start(out=wqk_sb[k][:, 640:1024],
                                    in_=wqk[128 * k:128 * (k + 1), 640:1024])
            for b in range(2, NB):
                for k in range(KCH):
                    nc.sync.dma_start(
                        out=xT_sb[k][:, 512 * b:512 * (b + 1)],
                        in_=xT[128 * k:128 * (k + 1), 512 * b:512 * (b + 1)])
            for c in range(DL // 128):
                nc.gpsimd.dma_start(out=wp_sb[c],
                                    in_=wp[128 * c:128 * (c + 1), :])

            # zero the unused kT halves before any S matmul reads them;
            # only block 0's columns are needed up front — later blocks'
            # zero-chunks ride along as filler with that block's QKV
            def kt2_zero_steps(b):
                steps = []
                for h in range(HL):
                    def ms(h=h, b=b):
                        z0, z1 = (64, 128) if h % 2 == 0 else (0, 64)
                        nc.gpsimd.memset(
                            kT2_sb[h][z0:z1, 512 * b:512 * (b + 1)], 0.0)
                    steps.append(ms)
                return steps

            for st in kt2_zero_steps(0):
                st()

            # ---- filler step machinery ----
            fill = deque()

            def qk_chain_steps(cc, b):
                st = {}
                steps = []
                for k in range(KCH):
                    def mm(k=k, cc=cc, b=b):
                        if k == 0:
                            st["ps"] = psF.tile([128, 512], F32, name="psF",
                                                tag="psF")
                        nc.tensor.matmul(
                            st["ps"],
                            wqk_sb[k][:, 128 * cc:128 * (cc + 1)],
                            xT_sb[k][:, 512 * b:512 * (b + 1)],
                            start=(k == 0), stop=(k == KCH - 1))
                    steps.append(mm)

                def bias(cc=cc, b=b):
                    ps = st["ps"]
                    tbs = slice(512 * b, 512 * (b + 1))
                    if cc < 4:
                        nc.vector.tensor_scalar_add(
                            out=qT_sb[cc][:, tbs], in0=ps,
                            scalar1=bqk_sb[:, cc:cc + 1])
                    else:
                        hA = 2 * (cc - 4)
                        nc.vector.tensor_scalar_add(
                            out=kT2_sb[hA][0:64, tbs], in0=ps[0:64, :],
                            scalar1=bqk_sb[0:64, cc:cc + 1])
                        nc.vector.tensor_scalar_add(
                            out=kT2_sb[hA + 1][64:128, tbs],
                            in0=ps[64:128, :],
                            scalar1=bqk_sb[64:128, cc:cc + 1])
                steps.append(bias)
                return steps

            def v_chain_steps(t):
                st = {}
                steps = []
                for k in range(KCH):
                    def mm(k=k, t=t):
                        if k == 0:
                            st["ps"] = psF.tile([128, 512], F32, name="psF",
                                                tag="psF")
                        nc.tensor.matmul(
                            st["ps"],
                            xT_sb[k][:, 128 * t:128 * (t + 1)],
                            wv_sb[k],
                            start=(k == 0), stop=(k == KCH - 1))
                    steps.append(mm)

                def cast(t=t):
                    va3 = vaug[t].rearrange("p (h c) -> p h c", c=HD + 1)
                    nc.vector.tensor_copy(
                        va3[:, :, 0:HD],
                        st["ps"].rearrange("p (h d) -> p h d", d=HD))
                    nc.gpsimd.memset(va3[:, :, HD], 1.0)
                steps.append(cast)
                return steps

            def oproj_steps(t, half):
                st = {}
                steps = []
                for c in range(DL // 128):
                    def mm(c=c, t=t, half=half):
                        if c == 0:
                            st["ps"] = psF.tile([128, 512], F32, name="psF",
                                                tag="psF")
                        nc.tensor.matmul(
                            st["ps"],
                            yT_sb[c][:, 128 * t:128 * (t + 1)],
                            wp_sb[c][:, 512 * half:512 * (half + 1)],
                            start=(c == 0), stop=(c == DL // 128 - 1))
                    steps.append(mm)

                def castdma(t=t, half=half):
                    oc = outp.tile([128, 512], F16, name="oc", tag="oc")
                    nc.vector.tensor_copy(oc, st["ps"])
                    eng = (nc.sync, nc.gpsimd)[(2 * t + half) % 2]
                    eng.dma_start(
                        out=out[128 * t:128 * (t + 1),
                                512 * half:512 * (half + 1)],
                        in_=oc)
                steps.append(castdma)
                return steps

            def qkv_block_steps(b):
                steps = kt2_zero_steps(b)
                steps += qk_chain_steps(0, b)
                steps += qk_chain_steps(4, b)
                for t in range(4 * b, 4 * b + 4):
                    steps += v_chain_steps(t)
                for cc in (1, 5, 2, 6, 3, 7):
                    steps += qk_chain_steps(cc, b)
                return steps

            def oproj_block_steps(q0):
                steps = []
                for t in range(4 * q0, 4 * q0 + 4):
                    for half in range(2):
                        steps += oproj_steps(t, half)
                return steps

            fill_pushed = [0]
            fill_popped = [0]

            def push(steps):
                fill.extend(steps)
                fill_pushed[0] += len(steps)
                return fill_pushed[0]

            def pop_fill(n):
                for _ in range(n):
                    if fill:
                        fill.popleft()()
                        fill_popped[0] += 1

            def drain_to(mark):
                while fill_popped[0] < mark and fill:
                    fill.popleft()()
                    fill_popped[0] += 1

            # ---- QKV for block 0: v + first head-pair only, the rest is
            # filler so attention(0) starts as early as possible ----
            for t in range(4):
                for st in v_chain_steps(t):
                    st()
            for st in qk_chain_steps(0, 0) + qk_chain_steps(4, 0):
                st()
            b0_marks = {}
            for cgrp, ccs in enumerate([(1, 5), (2, 6), (3, 7)]):
                for cc in ccs:
                    push(qk_chain_steps(cc, 0))
                b0_marks[cgrp + 1] = fill_pushed[0]

            # ---- attention pipeline over query blocks ----
            prev_qkv_mark = 0
            for q0 in range(NB):
                # all of block q0's QKV must be emitted before its attention
                drain_to(prev_qkv_mark)
                if q0 + 1 < NB:
                    prev_qkv_mark = push(qkv_block_steps(q0 + 1))
                # all out-projections run as filler inside attention(3),
                # whose exp load is largest; their DMAs round-robin the
                # three queues so the drain is spread out.
                if q0 == 3:
                    for qq in range(3):
                        push(oproj_block_steps(qq))

                ntiles = 4 * q0 + 4
                tiles_total = 4 * ntiles
                tiles_done = 0
                for c in range(4):
                    if q0 == 0 and c > 0:
                        drain_to(b0_marks[c])
                    ps_ys = [psY.tile([HD + 1, 512], F32, name="psY",
                                      tag="psY") for p in range(2)]
                    pend = deque()   # PV pipeline, depth 2

                    def emit_PV(t, es):
                        m = t - 4 * q0
                        lo = 128 * m if m > 0 else 0
                        for p in range(2):
                            nc.tensor.matmul(
                                ps_ys[p][:, lo:512],
                                vaug[t][:, (HD + 1) * (2 * c + p):
                                        (HD + 1) * (2 * c + p + 1)],
                                es[:, 512 * p + lo:512 * (p + 1)],
                                start=(t == 0), stop=(t == ntiles - 1))

                    for t in range(ntiles):
                        m = t - 4 * q0
                        lo = 128 * m if m > 0 else 0
                        ps_s = psS.tile([128, 1024], F32, name="psS",
                                        tag="psS")
                        for p in range(2):
                            nc.tensor.matmul(
                                ps_s[:, 512 * p + lo:512 * (p + 1)],
                                kT2_sb[2 * c + p][:, 128 * t:128 * (t + 1)],
                                qT_sb[c][:, 512 * q0 + lo:512 * (q0 + 1)],
                                start=True, stop=True)
                        if m >= 0:
                            seg = ps_s.rearrange("p (u f) -> p u f", u=2)
                            nc.vector.tensor_add(
                                seg[:, :, lo:lo + 128],
                                seg[:, :, lo:lo + 128],
                                tri_sb.unsqueeze(1).broadcast_to(
                                    [128, 2, 128]))
                        es = work.tile([128, 1024], F16, name="es", tag="es")
                        nc.scalar.activation(
                            out=es[:, lo:1024], in_=ps_s[:, lo:1024],
                            func=mybir.ActivationFunctionType.Exp)
                        # filler between S(t) and PV(t-3): PE covers the
                        # exp latency with independent projection matmuls.
                        # Keep a reserve in the last block to bridge the
                        # final norm before the last out-projection.
                        left = tiles_total - tiles_done
                        avail = len(fill) - (28 if q0 == NB - 1 else 0)
                        pace = (avail + left - 1) // left if avail > 0 else 0
                        # extra burst at the first PV of each c-group: it
                        # waits on the previous group's norm (psY reuse)
                        if t == 3:
                            pace += 3
                        pop_fill(pace)
                        tiles_done += 1
                        pend.append((t, es))
                        if len(pend) > 4:
                            emit_PV(*pend.popleft())
                    while pend:
                        emit_PV(*pend.popleft())

                    # normalize: y = yhat / denom (denom = ones-row of PV);
                    # p0/p1 interleaved so DVE and gpsimd pipeline
                    dn = [small.tile([1, 512], F32, name="dn", tag="dn")
                          for p in range(2)]
                    rc1 = [small.tile([1, 512], F32, name="rc1", tag="rc1")
                           for p in range(2)]
                    rcb = [small.tile([64, 512], F32, name="rcb", tag="rcb")
                           for p in range(2)]
                    for p in range(2):
                        nc.vector.tensor_copy(dn[p], ps_ys[p][HD:HD + 1, :])
                    for p in range(2):
                        nc.vector.reciprocal_approx_fast(rc1[p], dn[p])
                    for p in range(2):
                        nc.gpsimd.partition_broadcast(rcb[p], rc1[p])
                    for p in range(2):
                        nc.vector.tensor_mul(
                            yT_sb[c][64 * p:64 * (p + 1),
                                     512 * q0:512 * (q0 + 1)],
                            ps_ys[p][0:HD, :],
                            rcb[p])

            # drain the reserve (covers the final norm's latency), then
            # the last block's projection on the now-idle psS banks: four
            # wide chains with casts and DMAs pipelined behind them
            while fill:
                fill.popleft()()
            for t in range(4 * (NB - 1), 4 * NB):
                ps_o = psS.tile([128, 1024], F32, name="psS", tag="psS")
                for half in range(2):
                    for c in range(DL // 128):
                        nc.tensor.matmul(
                            ps_o[:, 512 * half:512 * (half + 1)],
                            yT_sb[c][:, 128 * t:128 * (t + 1)],
                            wp_sb[c][:, 512 * half:512 * (half + 1)],
                            start=(c == 0), stop=(c == DL // 128 - 1))
                oc = outp.tile([128, 1024], F16, name="ocw", tag="ocw")
                for half in range(2):
                    nc.vector.tensor_copy(
                        oc[:, 512 * half:512 * (half + 1)],
                        ps_o[:, 512 * half:512 * (half + 1)])
                    eng = (nc.sync, nc.gpsimd)[(2 * t + half) % 2]
                    eng.dma_start(
                        out=out[128 * t:128 * (t + 1),
                                512 * half:512 * (half + 1)],
                        in_=oc[:, 512 * half:512 * (half + 1)])

    nc.finalize()
    return nc


def _enable_trace_hooks():
    """Inject antenv.axon_hooks + no-op artifact upload so that
    run_bass_kernel_spmd(trace=True) works under axon in this image."""
    import types
    import antenv

    if "antenv.axon_hooks" not in sys.modules:
        mod = types.ModuleType("antenv.axon_hooks")
        state = {"hook": None}
        mod.set_axon_ntff_profile_hook = lambda h: state.__setitem__("hook", h)
        mod.get_axon_ntff_profile_hook = lambda: state["hook"]
        sys.modules["antenv.axon_hooks"] = mod
        antenv.axon_hooks = mod
        from trn_agent_boot.trn_boot import _ntff_profile_via_ctypes

        mod.set_axon_ntff_profile_hook(
            _ntff_profile_via_ctypes("/opt/axon/libaxon_pjrt.so"))
    from concourse import bass_utils as bu

    bu.upload_artifacts = lambda tmpdir: str(tmpdir)


def kernel(x, w_attn, b_attn, w_proj, b_proj, _trace=False):
    x = np.asarray(x)
    w_attn = np.asarray(w_attn)
    b_attn = np.asarray(b_attn)
    w_proj = np.asarray(w_proj)
    b_proj = np.asarray(b_proj)

    if "nc" not in _cache:
        _cache["nc"] = _build()
    nc = _cache["nc"]

    scale = 1.0 / np.sqrt(HD)
    f16 = np.float16
    tri = np.where(np.arange(128)[:, None] <= np.arange(128)[None, :],
                   np.float32(0.0), np.float32(NEG)).astype(np.float32)

    in_maps = []
    for core in range(8):
        b, hg = core // 2, core % 2
        qs = slice(hg * DL, (hg + 1) * DL)
        ks = slice(D + hg * DL, D + (hg + 1) * DL)
        wq = (w_attn[:, qs] * scale).astype(f16)
        wk = w_attn[:, ks].astype(f16)
        wqk_host = np.concatenate([wq, wk], axis=1)
        bqk_host = np.concatenate(
            [b_attn[qs] * scale, b_attn[ks]]).astype(np.float32)
        vs = slice(2 * D + hg * DL, 2 * D + (hg + 1) * DL)
        in_maps.append({
            "xT": np.ascontiguousarray(x[b].T).astype(f16),
            "wqk": np.ascontiguousarray(wqk_host),
            "bqk": np.ascontiguousarray(bqk_host.reshape(8, 128).T),
            "wv": np.ascontiguousarray(w_attn[:, vs]).astype(f16),
            "wp": np.ascontiguousarray(
                w_proj[hg * DL:(hg + 1) * DL, :]).astype(f16),
            "tri": tri,
        })

    kwargs = {}
    if _trace:
        _enable_trace_hooks()
        kwargs = dict(trace=True, trace_cores=[0])
    res = run_bass_kernel_spmd(nc, in_maps, core_ids=list(range(8)), **kwargs)

    # host epilogue: sum TP partials, add b_proj and the folded v-bias term
    bias_total = (b_attn[2 * D:].astype(np.float32) @
                  w_proj.astype(np.float32)) + b_proj.astype(np.float32)
    outp = np.empty((B, T, D), np.float32)
    for b in range(B):
        outp[b] = (res.results[2 * b]["out"].astype(np.float32) +
                   res.results[2 * b + 1]["out"].astype(np.float32))
    outp += bias_total

    if _trace:
        print(f"HW exec time: {res.exec_time_ns} ns")
    return outp


# revision 36
# speedup vs baseline: 1.0199x; 1.0199x over previous
"""Causal self-attention (B=4, T=2048, D=1024, H=16) on 8 trn2 NeuronCores.

Sharding: Megatron-style tensor parallel over heads (TP=2) x data parallel
over batch (DP=4). Core c handles batch c//2 and head-group c%2 (8 heads).
Each core computes its QKV projection slice, causal attention for its 8
heads, and a partial output projection; the host sums the two TP partials
per batch and adds b_proj (plus the folded v-bias contribution).

Schedule: single software-pipelined stream.  Attention for query block q0
is exp-rate-limited on the scalar engine, so independent projection
matmuls (QKV of block q0+1, output projection of earlier blocks) are
interleaved into the attention instruction stream as PE filler.  The v
bias is folded into a host-side output correction (attn rows sum to 1),
and the output partials are written as fp16 to halve the drain DMA.
"""
import sys
from collections import deque

sys.path.insert(0, "/opt/trn_rl_repo")

import numpy as np
import ml_dtypes

import concourse.bass as bass
import concourse.tile as tile
from concourse import bacc, mybir
from concourse.bass_utils import run_bass_kernel_spmd

B, T, D, H = 4, 2048, 1024, 16
HD = 64            # head dim
HL = 8             # heads per core (TP=2)
DL = HL * HD       # 512 local qkv width
KCH = D // 128     # 8 contraction chunks
TCH = T // 128     # 16 token tiles of 128
NB = T // 512      # 4 query blocks of 512
F16 = mybir.dt.float16
F32 = mybir.dt.float32
NEG = -1.0e30

_cache = {}


def _build():
    nc = bacc.Bacc("TRN2", target_bir_lowering=False, num_devices=8)

    xT = nc.dram_tensor("xT", [D, T], F16, kind="ExternalInput")
    wqk = nc.dram_tensor("wqk", [D, 2 * DL], F16, kind="ExternalInput")
    bqk = nc.dram_tensor("bqk", [128, 2 * DL // 128], F32, kind="ExternalInput")
    wv = nc.dram_tensor("wv", [D, DL], F16, kind="ExternalInput")
    wp = nc.dram_tensor("wp", [DL, D], F16, kind="ExternalInput")
    tri = nc.dram_tensor("tri", [128, 128], F32, kind="ExternalInput")
    out = nc.dram_tensor("out", [T, D], F16, kind="ExternalOutput")

    with tile.TileContext(nc) as tc:
        with (
            tc.tile_pool(name="const", bufs=1) as const,
            tc.tile_pool(name="acts", bufs=1) as acts,
            tc.tile_pool(name="work", bufs=5) as work,
            tc.tile_pool(name="small", bufs=6) as small,
            tc.tile_pool(name="outp", bufs=4) as outp,
            tc.tile_pool(name="psS", bufs=2, space="PSUM") as psS,
            tc.tile_pool(name="psY", bufs=2, space="PSUM") as psY,
            tc.tile_pool(name="psF", bufs=2, space="PSUM") as psF,
        ):
            # ---- SBUF residents ----
            xT_sb = [const.tile([128, T], F16, name=f"xT{k}", tag=f"xT{k}")
                     for k in range(KCH)]
            wqk_sb = [const.tile([128, 2 * DL], F16, name=f"wqk{k}",
                                 tag=f"wqk{k}") for k in range(KCH)]
            wv_sb = [const.tile([128, DL], F16, name=f"wv{k}", tag=f"wv{k}")
                     for k in range(KCH)]
            wp_sb = [const.tile([128, D], F16, name=f"wp{c}", tag=f"wp{c}")
                     for c in range(DL // 128)]
            bqk_sb = const.tile([128, 2 * DL // 128], F32)
            tri_sb = const.tile([128, 128], F32)

            qT_sb = [acts.tile([128, T], F16, name=f"qT{c}", tag=f"qT{c}")
                     for c in range(4)]
            # kT per head, zero-padded to K=128: head 2c in partitions 0:64
            # of kT2[2c], head 2c+1 in partitions 64:128 of kT2[2c+1]; the
            # other half stays zero so S matmuls run full-K (no row-group
            # masking -> PE clock stays at full rate).
            kT2_sb = [acts.tile([128, T], F16, name=f"kT2h{h}", tag=f"kT2h{h}")
                      for h in range(HL)]
            vaug = [acts.tile([128, HL * (HD + 1)], F16, name=f"va{t}",
                              tag=f"va{t}") for t in range(TCH)]
            yT_sb = [acts.tile([128, T], F16, name=f"yT{c}", tag=f"yT{c}")
                     for c in range(4)]

            # ---- input DMAs on 3 queues, in consumption order ----
            # The v chains of block 0 are the first consumers: they need
            # xT t-tile 0 (all k chunks) + wv, so those go first at fine
            # granularity.  sync: xT; scalar: wqk; gpsimd: wv/bqk/tri/wp.
            for t in range(4):
                for k in range(KCH):
                    nc.sync.dma_start(
                        out=xT_sb[k][:, 128 * t:128 * (t + 1)],
                        in_=xT[128 * k:128 * (k + 1), 128 * t:128 * (t + 1)])
            for k in range(KCH):
                nc.gpsimd.dma_start(out=wv_sb[k],
                                    in_=wv[128 * k:128 * (k + 1), :])
                nc.scalar.dma_start(out=wqk_sb[k][:, 0:128],
                                    in_=wqk[128 * k:128 * (k + 1), 0:128])
                nc.scalar.dma_start(out=wqk_sb[k][:, 512:640],
                                    in_=wqk[128 * k:128 * (k + 1), 512:640])
                if k == 3:
                    nc.gpsimd.dma_start(out=bqk_sb, in_=bqk[:, :])
                    nc.gpsimd.dma_start(out=tri_sb, in_=tri[:, :])
            for k in range(KCH):
                nc.sync.dma_start(out=xT_sb[k][:, 512:1024],
                                  in_=xT[128 * k:128 * (k + 1), 512:1024])
                nc.scalar.dma_start(out=wqk_sb[k][:, 128:512],
                                    in_=wqk[128 * k:128 * (k + 1), 128:512])
                nc.scalar.dma_start(out=wqk_sb[k][:, 640:1024],
                                    in_=wqk[128 * k:128 * (k + 1), 640:1024])
            for b in range(2, NB):
                for k in range(KCH):
                    nc.sync.dma_start(
                        out=xT_sb[k][:, 512 * b:512 * (b + 1)],
                        in_=xT[128 * k:128 * (k + 1), 512 * b:512 * (b + 1)])
            for c in range(DL // 128):
                nc.gpsimd.dma_start(out=wp_sb[c],
                                    in_=wp[128 * c:128 * (c + 1), :])

            # zero the unused kT halves before any S matmul reads them;
            # only block 0's columns are needed up front — later blocks'
            # zero-chunks ride along as filler with that block's QKV
            def kt2_zero_steps(b):
                steps = []
                for h in range(HL):
                    def ms(h=h, b=b):
                        z0, z1 = (64, 128) if h % 2 == 0 else (0, 64)
                        nc.gpsimd.memset(
                            kT2_sb[h][z0:z1, 512 * b:512 * (b + 1)], 0.0)
                    steps.append(ms)
                return steps

            for st in kt2_zero_steps(0):
                st()

            # ---- filler step machinery ----
            fill = deque()

            def qk_chain_steps(cc, b):
                st = {}
                steps = []
                for k in range(KCH):
                    def mm(k=k, cc=cc, b=b):
                        if k == 0:
                            st["ps"] = psF.tile([128, 512], F32, name="psF",
                                                tag="psF")
                        nc.tensor.matmul(
                            st["ps"],
                            wqk_sb[k][:, 128 * cc:128 * (cc + 1)],
                            xT_sb[k][:, 512 * b:512 * (b + 1)],
                            start=(k == 0), stop=(k == KCH - 1))
                    steps.append(mm)

                def bias(cc=cc, b=b):
                    ps = st["ps"]
                    tbs = slice(512 * b, 512 * (b + 1))
                    if cc < 4:
                        nc.vector.tensor_scalar_add(
                            out=qT_sb[cc][:, tbs], in0=ps,
                            scalar1=bqk_sb[:, cc:cc + 1])
                    else:
                        hA = 2 * (cc - 4)
                        nc.vector.tensor_scalar_add(
                            out=kT2_sb[hA][0:64, tbs], in0=ps[0:64, :],
                            scalar1=bqk_sb[0:64, cc:cc + 1])
                        nc.vector.tensor_scalar_add(
                            out=kT2_sb[hA + 1][64:128, tbs],
                            in0=ps[64:128, :],
                            scalar1=bqk_sb[64:128, cc:cc + 1])
                steps.append(bias)
                return steps

            def v_chain_steps(t):
                st = {}
                steps = []
                for k in range(KCH):
                    def mm(k=k, t=t):
                        if k == 0:
                            st["ps"] = psF.tile([128, 512], F32, name="psF",
                                                tag="psF")
                        nc.tensor.matmul(
                            st["ps"],
                            xT_sb[k][:, 128 * t:128 * (t + 1)],
                            wv_sb[k],
                            start=(k == 0), stop=(k == KCH - 1))
                    steps.append(mm)

                def cast(t=t):
                    va3 = vaug[t].rearrange("p (h c) -> p h c", c=HD + 1)
                    nc.vector.tensor_copy(
                        va3[:, :, 0:HD],
                        st["ps"].rearrange("p (h d) -> p h d", d=HD))
                    nc.gpsimd.memset(va3[:, :, HD], 1.0)
                steps.append(cast)
                return steps

            def oproj_steps(t, half):
                st = {}
                steps = []
                for c in range(DL // 128):
                    def mm(c=c, t=t, half=half):
                        if c == 0:
                            st["ps"] = psF.tile([128, 512], F32, name="psF",
                                                tag="psF")
                        nc.tensor.matmul(
                            st["ps"],
                            yT_sb[c][:, 128 * t:128 * (t + 1)],
                            wp_sb[c][:, 512 * half:512 * (half + 1)],
                            start=(c == 0), stop=(c == DL // 128 - 1))
                    steps.append(mm)

                def castdma(t=t, half=half):
                    oc = outp.tile([128, 512], F16, name="oc", tag="oc")
                    nc.vector.tensor_copy(oc, st["ps"])
                    eng = (nc.sync, nc.gpsimd)[(2 * t + half) % 2]
                    eng.dma_start(
                        out=out[128 * t:128 * (t + 1),
                                512 * half:512 * (half + 1)],
                        in_=oc)
                steps.append(castdma)
                return steps

            def qkv_block_steps(b):
                steps = kt2_zero_steps(b)
                steps += qk_chain_steps(0, b)
                steps += qk_chain_steps(4, b)
                for t in range(4 * b, 4 * b + 4):
                    steps += v_chain_steps(t)
                for cc in (1, 5, 2, 6, 3, 7):
                    steps += qk_chain_steps(cc, b)
                return steps

            def oproj_block_steps(q0):
                steps = []
                for t in range(4 * q0, 4 * q0 + 4):
                    for half in range(2):
                        steps += oproj_steps(t, half)
                return steps

            fill_pushed = [0]
            fill_popped = [0]

            def push(steps):
                fill.extend(steps)
                fill_pushed[0] += len(steps)
                return fill_pushed[0]

            def pop_fill(n):
                for _ in range(n):
                    if fill:
                        fill.popleft()()
                        fill_popped[0] += 1

            def drain_to(mark):
                while fill_popped[0] < mark and fill:
                    fill.popleft()()
                    fill_popped[0] += 1

            # ---- QKV for block 0: v + first head-pair only, the rest is
            # filler so attention(0) starts as early as possible ----
            for t in range(4):
                for st in v_chain_steps(t):
                    st()
            for st in qk_chain_steps(0, 0) + qk_chain_steps(4, 0):
                st()
            b0_marks = {}
            for cgrp, ccs in enumerate([(1, 5), (2, 6), (3, 7)]):
                for cc in ccs:
                    push(qk_chain_steps(cc, 0))
                b0_marks[cgrp + 1] = fill_pushed[0]

            # ---- attention pipeline over query blocks ----
            prev_qkv_mark = 0
            for q0 in range(NB):
                # all of block q0's QKV must be emitted before its attention
                drain_to(prev_qkv_mark)
                if q0 + 1 < NB:
                    prev_qkv_mark = push(qkv_block_steps(q0 + 1))
                # all out-projections run as filler inside attention(3),
                # whose exp load is largest; their DMAs round-robin the
                # three queues so the drain is spread out.
                if q0 == 3:
                    for qq in range(3):
                        push(oproj_block_steps(qq))

                ntiles = 4 * q0 + 4
                tiles_total = 4 * ntiles
                tiles_done = 0
                for c in range(4):
                    if q0 == 0 and c > 0:
                        drain_to(b0_marks[c])
                    ps_ys = [psY.tile([HD + 1, 512], F32, name="psY",
                                      tag="psY") for p in range(2)]
                    pend = deque()   # PV pipeline, depth 2

                    def emit_PV(t, es):
                        m = t - 4 * q0
                        lo = 128 * m if m > 0 else 0
                        for p in range(2):
                            nc.tensor.matmul(
                                ps_ys[p][:, lo:512],
                                vaug[t][:, (HD + 1) * (2 * c + p):
                                        (HD + 1) * (2 * c + p + 1)],
                                es[:, 512 * p + lo:512 * (p + 1)],
                                start=(t == 0), stop=(t == ntiles - 1))

                    for t in range(ntiles):
                        m = t - 4 * q0
                        lo = 128 * m if m > 0 else 0
                        ps_s = psS.tile([128, 1024], F32, name="psS",
                                        tag="psS")
                        for p in range(2):
                            nc.tensor.matmul(
                                ps_s[:, 512 * p + lo:512 * (p + 1)],
                                kT2_sb[2 * c + p][:, 128 * t:128 * (t + 1)],
                                qT_sb[c][:, 512 * q0 + lo:512 * (q0 + 1)],
                                start=True, stop=True)
                        if m >= 0:
                            seg = ps_s.rearrange("p (u f) -> p u f", u=2)
                            nc.vector.tensor_add(
                                seg[:, :, lo:lo + 128],
                                seg[:, :, lo:lo + 128],
                                tri_sb.unsqueeze(1).broadcast_to(
                                    [128, 2, 128]))
                        es = work.tile([128, 1024], F16, name="es", tag="es")
                        nc.scalar.activation(
                            out=es[:, lo:1024], in_=ps_s[:, lo:1024],
                            func=mybir.ActivationFunctionType.Exp)
                        # filler between S(t) and PV(t-3): PE covers the
                        # exp latency with independent projection matmuls.
                        # Keep a reserve in the last block to bridge the
                        # final norm before the last out-projection.
                        left = tiles_total - tiles_done
                        avail = len(fill) - (28 if q0 == NB - 1 else 0)
                        pace = (avail + left - 1) // left if avail > 0 else 0
                        # extra burst at the first PV of each c-group: it
                        # waits on the previous group's norm (psY reuse)
                        if t == 3:
                            pace += 3
                        pop_fill(pace)
                        tiles_done += 1
                        pend.append((t, es))
                        if len(pend) > 4:
                            emit_PV(*pend.popleft())
                    while pend:
                        emit_PV(*pend.popleft())

                    # normalize: y = yhat / denom (denom = ones-row of PV);
                    # p0/p1 interleaved so DVE and gpsimd pipeline
                    dn = [small.tile([1, 512], F32, name="dn", tag="dn")
                          for p in range(2)]
                    rc1 = [small.tile([1, 512], F32, name="rc1", tag="rc1")
                           for p in range(2)]
                    rcb = [small.tile([64, 512], F32, name="rcb", tag="rcb")
                           for p in range(2)]
                    for p in range(2):
                        nc.vector.tensor_copy(dn[p], ps_ys[p][HD:HD + 1, :])
                    for p in range(2):
                        nc.vector.reciprocal_approx_fast(rc1[p], dn[p])
                    for p in range(2):
                        nc.gpsimd.partition_broadcast(rcb[p], rc1[p])
                    for p in range(2):
                        nc.vector.tensor_mul(
                            yT_sb[c][64 * p:64 * (p + 1),
                                     512 * q0:512 * (q0 + 1)],
                            ps_ys[p][0:HD, :],
                            rcb[p])

            # last block's projection on the now-idle psS banks.  The c<3
            # partial chains don't depend on the final c-group's norm, so
            # two of them plus the fill reserve bridge the norm's latency
            # and keep the PE clock from parking; the c=3 finishers, casts
            # and DMAs pipeline behind.
            ps_map = {}

            def wide_partial(t):
                ps_o = psS.tile([128, 1024], F32, name="psS", tag="psS")
                ps_map[t] = ps_o
                for half in range(2):
                    for c in range(3):
                        nc.tensor.matmul(
                            ps_o[:, 512 * half:512 * (half + 1)],
                            yT_sb[c][:, 128 * t:128 * (t + 1)],
                            wp_sb[c][:, 512 * half:512 * (half + 1)],
                            start=(c == 0), stop=False)

            def wide_finish(t):
                ps_o = ps_map[t]
                for half in range(2):
                    nc.tensor.matmul(
                        ps_o[:, 512 * half:512 * (half + 1)],
                        yT_sb[3][:, 128 * t:128 * (t + 1)],
                        wp_sb[3][:, 512 * half:512 * (half + 1)],
                        start=False, stop=True)
                oc = outp.tile([128, 1024], F16, name="ocw", tag="ocw")
                for half in range(2):
                    nc.vector.tensor_copy(
                        oc[:, 512 * half:512 * (half + 1)],
                        ps_o[:, 512 * half:512 * (half + 1)])
                    eng = (nc.sync, nc.gpsimd)[(2 * t + half) % 2]
                    eng.dma_start(
                        out=out[128 * t:128 * (t + 1),
                                512 * half:512 * (half + 1)],
                        in_=oc[:, 512 * half:512 * (half + 1)])

            t0 = 4 * (NB - 1)
            wide_partial(t0)
            wide_partial(t0 + 1)
            while fill:
                fill.popleft()()
            wide_finish(t0)
            wide_partial(t0 + 2)
            wide_finish(t0 + 1)
            wide_partial(t0 + 3)
            wide_finish(t0 + 2)
            wide_finish(t0 + 3)

    nc.finalize()
    return nc


def _enable_trace_hooks():
    """Inject antenv.axon_hooks + no-op artifact upload so that
    run_bass_kernel_spmd(trace=True) works under axon in this image."""
    import types
    import antenv

    if "antenv.axon_hooks" not in sys.modules:
        mod = types.ModuleType("antenv.axon_hooks")
        state = {"hook": None}
        mod.set_axon_ntff_profile_hook = lambda h: state.__setitem__("hook", h)
        mod.get_axon_ntff_profile_hook = lambda: state["hook"]
        sys.modules["antenv.axon_hooks"] = mod
        antenv.axon_hooks = mod
        from trn_agent_boot.trn_boot import _ntff_profile_via_ctypes

        mod.set_axon_ntff_profile_hook(
            _ntff_profile_via_ctypes("/opt/axon/libaxon_pjrt.so"))
    from concourse import bass_utils as bu

    bu.upload_artifacts = lambda tmpdir: str(tmpdir)


def kernel(x, w_attn, b_attn, w_proj, b_proj, _trace=False):
    x = np.asarray(x)
    w_attn = np.asarray(w_attn)
    b_attn = np.asarray(b_attn)
    w_proj = np.asarray(w_proj)
    b_proj = np.asarray(b_proj)

    if "nc" not in _cache:
        _cache["nc"] = _build()
    nc = _cache["nc"]

    scale = 1.0 / np.sqrt(HD)
    f16 = np.float16
    tri = np.where(np.arange(128)[:, None] <= np.arange(128)[None, :],
                   np.float32(0.0), np.float32(NEG)).astype(np.float32)

    in_maps = []
    for core in range(8):
        b, hg = core // 2, core % 2
        qs = slice(hg * DL, (hg + 1) * DL)
        ks = slice(D + hg * DL, D + (hg + 1) * DL)
        wq = (w_attn[:, qs] * scale).astype(f16)
        wk = w_attn[:, ks].astype(f16)
        wqk_host = np.concatenate([wq, wk], axis=1)
        bqk_host = np.concatenate(
            [b_attn[qs] * scale, b_attn[ks]]).astype(np.float32)
        vs = slice(2 * D + hg * DL, 2 * D + (hg + 1) * DL)
        in_maps.append({
            "xT": np.ascontiguousarray(x[b].T).astype(f16),
            "wqk": np.ascontiguousarray(wqk_host),
            "bqk": np.ascontiguousarray(bqk_host.reshape(8, 128).T),
            "wv": np.ascontiguousarray(w_attn[:, vs]).astype(f16),
            "wp": np.ascontiguousarray(
                w_proj[hg * DL:(hg + 1) * DL, :]).astype(f16),
            "tri": tri,
        })

    kwargs = {}
    if _trace:
        _enable_trace_hooks()
        kwargs = dict(trace=True, trace_cores=[0])
    res = run_bass_kernel_spmd(nc, in_maps, core_ids=list(range(8)), **kwargs)

    # host epilogue: sum TP partials, add b_proj and the folded v-bias term
    bias_total = (b_attn[2 * D:].astype(np.float32) @
                  w_proj.astype(np.float32)) + b_proj.astype(np.float32)
    outp = np.empty((B, T, D), np.float32)
    for b in range(B):
        outp[b] = (res.results[2 * b]["out"].astype(np.float32) +
                   res.results[2 * b + 1]["out"].astype(np.float32))
    outp += bias_total

    if _trace:
        print(f"HW exec time: {res.exec_time_ns} ns")
    return outp
